# revision 22
# baseline (speedup 1.0000x reference)
"""Trainium2 Bass kernel for the soft-logic-gate CA problem.

Math (per sample, grid 128x128, 4 layers):
  state' = clip( sum_m sigmoid(tg[l,m]) * prod_j g(bit_j(m), tap_j), 0, 1 )
  taps: A=state[x,y], B=state[x,y+1], C=state[x+1,y], D=state[x+1,y+1] (periodic)
  g(0,t)=1-t, g(1,t)=t;  m = bA*8 + bB*4 + bC*2 + bD.

This is 4-D multilinear interpolation of the 16 gate maps at corner
(A,B,C,D).  Sigmoided gates are converted to multilinear-polynomial
coefficients with an in-place Moebius transform (c[m] -= c[m-bit]) and
each layer is evaluated with a Horner butterfly of fused tensor_tensor
ops, contracting A first (its tap needs no shift), then B, C, D:
  u_i = c[i] + c[8+i]*A ; v_j = u_j + u_{4+j}*B ; w_k = ... ; s = w0 + w1*D

Sharding: batch 32 -> 8 cores x 4 samples (gates replicated).
Layout per core: partition = grid row (128), free = (sample b:4, col y:128).
Coefficients stay compact (one copy); batch broadcast via stride-0 APs.
Row shifts (x+1) run on the idle TensorE as a permutation matmul into
PSUM, then ScalarE copies back to SBUF; col shifts (y+1) are cheap
same-partition DMAs.  GpSimd is left idle on purpose: its SBUF port is
shared with VectorE and concurrent use slows DVE ~4-6x (measured).

Compute dtype is fp16 (DVE 2x_1P mode on unit-stride ops; ~1.7e-3 rel
err vs fp32 reference, measured).  Layer 0 reads only 8 gate maps (the
initial state has x in even columns, 0 in odd), and layer 3 computes
only even output columns; both use de-interleaved compact coefficient
blocks so every hot op keeps unit stride.
"""

import numpy as np

import concourse.bacc as bacc
import concourse.mybir as mybir
from concourse.tile import TileContext
from concourse.bass_utils import run_bass_kernel_spmd

F32 = mybir.dt.float32
DT = mybir.dt.float16  # compute dtype (float32 also works)
AL = mybir.AluOpType
P = 128          # partitions = grid rows
B = 4            # samples per core
Y = 128          # grid cols
K = 64           # x cols (even grid cols)
L = 4            # layers
M = 16           # gate combos
N_CORES = 8


def _emit(tc, nc, x_ap, tg_ap, ps_ap, out_ap_d):
    sync, vec, act, ten = nc.sync, nc.vector, nc.scalar, nc.tensor
    SIG = mybir.ActivationFunctionType.Sigmoid

    def colshift(dst, src, w, engines):
        # per sample block of width w: dst[., b, y] = src[., b, (y+1) % w]
        d = dst[:].rearrange("p (b y) -> p b y", b=B)
        s = src[:].rearrange("p (b y) -> p b y", b=B)
        engines[0].dma_start(out=d[:, :, 0 : w - 1], in_=s[:, :, 1:w])
        engines[1].dma_start(out=d[:, :, w - 1 : w], in_=s[:, :, 0:1])

    def bcast_c(c):  # coeff (p, n, w) -> (p, n, B, w)
        n, w = c.shape[1], c.shape[2]
        return c.unsqueeze(2).broadcast_to((P, n, B, w))

    def bcast_t(t, n):  # tap (p, B, w) -> (p, n, B, w)
        w = t.shape[2]
        return t.unsqueeze(1).broadcast_to((P, n, B, w))

    def clamp(out_ap, in_ap):
        vec.tensor_scalar(
            out=out_ap, in0=in_ap, scalar1=0.0, scalar2=1.0, op0=AL.max, op1=AL.min
        )

    with (
        tc.tile_pool(name="coef", bufs=1) as pc,
        tc.tile_pool(name="io", bufs=2) as pio,
        tc.tile_pool(name="st", bufs=2) as pst,
        tc.tile_pool(name="wk", bufs=1) as pwk,
        tc.tile_pool(name="ps", bufs=2, space="PSUM") as pps,
    ):
        # preload the sigmoid ACT table while DMAs run
        scr = pwk.tile([P, 1], F32, tag="scr")
        vec.memset(scr[:], 0.0)
        act.activation(out=scr[:], in_=scr[:], func=SIG)

        # ---- loads (tg0 first: it gates the layer-0 coefficient chain) ----
        tw = pc.tile([P, L * M * Y], DT, tag="tw")  # coeffs, all layers
        tga = tg_ap

        def twl(l):
            return tw[:, l * M * Y : (l + 1) * M * Y]

        # layer 0 needs only 8 gate maps: {0,2,8,10} (even-y outputs) and
        # {0,1,4,5} (odd-y outputs); m = a*8 + bb*4 + c*2 + d.
        # tgraw0 = [4 even-set maps (a,c) | 4 odd-set maps (bb,d)] x y
        tgraw0 = pio.tile([P, 8 * Y], F32, tag="tgraw0")
        tg0t = tga[0].transpose([1, 0, 2])  # (P, M, Y)
        tg0m = tg0t.rearrange("p (a bb c d) y -> p a bb c d y", a=2, bb=2, c=2, d=2)
        g0e = tgraw0[:, 0 : 4 * Y].rearrange("p (a c y) -> p a c y", a=2, c=2)
        g0o = tgraw0[:, 4 * Y : 8 * Y].rearrange("p (bb d y) -> p bb d y", bb=2, d=2)
        for i in (0, 1):  # DMA APs allow at most partition + 3 dims
            sync.dma_start(out=g0e[:, i], in_=tg0m[:, i, 0, :, 0, :])
        for i in (0, 1):
            sync.dma_start(out=g0o[:, i], in_=tg0m[:, 0, i, 0, :, :])
        # layer-1 gates are also critical-path: split across the scalar and
        # gpsimd DMA rings so they don't queue behind anything big
        half = 8 * Y
        tgraw1 = pio.tile([P, M * Y], F32, tag="tgraw")
        tg1t = tga[1].transpose([1, 0, 2])
        psh = pwk.tile([P, P], DT, tag="psh")
        nc.scalar.dma_start(out=psh[:], in_=ps_ap)
        nc.scalar.dma_start(
            out=tgraw1[:, 0:half].rearrange("p (m y) -> p m y", m=8),
            in_=tg1t[:, 0:8],
        )

        # layer-0 taps, loaded straight from DRAM with casting gpsimd DMAs:
        # X, Xc = colshift(X) (as rotated DRAM slices); Xr/Xrc via PE rowshift
        xt = x_ap.transpose([1, 0, 2])  # (P, B, K)
        X = pwk.tile([P, B * K], DT, tag="X")
        Xc = pwk.tile([P, B * K], DT, tag="Xc")
        Xv = X[:].rearrange("p (b k) -> p b k", b=B)
        Xcv = Xc[:].rearrange("p (b k) -> p b k", b=B)
        nc.gpsimd.dma_start(out=Xv, in_=xt)
        nc.gpsimd.dma_start(out=Xcv[:, :, 0 : K - 1], in_=xt[:, :, 1:K])
        nc.gpsimd.dma_start(out=Xcv[:, :, K - 1 : K], in_=xt[:, :, 0:1])
        nc.gpsimd.dma_start(
            out=tgraw1[:, half : 2 * half].rearrange("p (m y) -> p m y", m=8),
            in_=tg1t[:, 8:16],
        )
        pXr = pps.tile([P, B * K], F32, tag="pXr")
        pXrc = pps.tile([P, B * K], F32, tag="pXrc")
        ten.matmul(pXr[:], psh[:], X[:], start=True, stop=True)
        ten.matmul(pXrc[:], psh[:], Xc[:], start=True, stop=True)
        Xr = pwk.tile([P, B * K], DT, tag="Xr")
        Xrc = pwk.tile([P, B * K], DT, tag="Xrc")
        vec.tensor_copy(out=Xr[:], in_=pXr[:])
        # (Xrc copy is emitted after the even-half eval: Vector runs in
        # program order and the even half must not wait for pXrc)

        # ---- layer-0 coefficients: two compact 4-map blocks ----
        #   ce = twl0[0:256]   = [c0,c2,c8,c10] x k    (order (a,c))
        #   co = twl0[256:512] = [c0,c1,c4,c5]  x k    (order (bb,d))
        t0e = g0e.rearrange("p a c (k t) -> p a c k t", t=2)
        t0o = g0o.rearrange("p bb d (k t) -> p bb d k t", t=2)
        ce = twl(0)[:, 0:256]
        co = twl(0)[:, 256:512]
        ce4 = ce.rearrange("p (a c k) -> p a c k", a=2, c=2)
        co4 = co.rearrange("p (bb d k) -> p bb d k", bb=2, d=2)
        act.activation(out=ce4, in_=t0e[:, :, :, :, 0], func=SIG)
        act.activation(out=co4, in_=t0o[:, :, :, :, 1], func=SIG)
        # 2-D Moebius on each block (2 fused subtract passes each)
        for blk, n4 in ((ce, ce4), (co, co4)):
            d_ = n4[:, :, 1]
            s_ = n4[:, :, 0]
            vec.tensor_tensor(out=d_, in0=d_, in1=s_, op=AL.subtract)
            hi = blk.rearrange("p (h q) -> p h q", h=2)
            vec.tensor_tensor(
                out=hi[:, 1], in0=hi[:, 1], in1=hi[:, 0], op=AL.subtract
            )

        # remaining layers: full sigmoid (layer 3 de-interleaved to even-y);
        # layer 1 in two halves so its Moebius pipeline starts earlier
        act.activation(out=twl(1)[:, 0:half], in_=tgraw1[:, 0:half], func=SIG)
        act.activation(out=twl(1)[:, half : 2 * half], in_=tgraw1[:, half : 2 * half], func=SIG)
        for l in (2, 3):
            tgraw = pio.tile([P, M * Y], F32, tag="tgraw")
            sync.dma_start(
                out=tgraw[:].rearrange("p (m y) -> p m y", m=M),
                in_=tga[l].transpose([1, 0, 2]),
            )
            if l < 3:
                act.activation(out=twl(l), in_=tgraw[:], func=SIG)
            else:
                tr = tgraw[:].rearrange("p (m k t) -> p m k t", m=M, t=2)
                c3 = twl(3)[:, 0 : M * K].rearrange("p (m k) -> p m k", m=M)
                act.activation(out=c3, in_=tr[:, :, :, 0], func=SIG)

        def mobius_full(block, w):
            # block: (p, 16*w) coeff AP; in-place c[m] -= c[m-bit] per bit
            for s in (1, 2, 4, 8):
                hi = 8 // s
                v = block.rearrange(
                    "p (hi two lo y) -> p hi two lo y", hi=hi, two=2, lo=s, y=w
                )
                vec.tensor_tensor(
                    out=v[:, :, 1], in0=v[:, :, 1], in1=v[:, :, 0], op=AL.subtract
                )

        # ---- layer 0 eval (A-first 2-D interp) ----
        st1 = pst.tile([P, B * Y], DT, tag="state")
        st1v = st1[:].rearrange("p (b k t) -> p b k t", b=B, t=2)
        ue = pwk.tile([P, 2 * B * K], DT, tag="ue")
        te = pwk.tile([P, B * K], DT, tag="te")
        uev = ue[:].rearrange("p (s b k) -> p s b k", s=2, b=B)
        tev = te[:].rearrange("p (b k) -> p b k", b=B)
        bv = lambda t: t[:].rearrange("p (b k) -> p b k", b=B)

        def l0_half(par, cpair, t_in, t_out):
            # s = (c_lo0 + c_hi0 * t_in) + t_out * (c_lo1 + c_hi1 * t_in)
            cp = cpair.rearrange("p (h s k) -> p h s k", h=2, s=2)
            vec.tensor_tensor(out=uev, in0=bcast_c(cp[:, 1]), in1=bcast_t(t_in, 2), op=AL.mult)
            vec.tensor_tensor(out=uev, in0=uev, in1=bcast_c(cp[:, 0]), op=AL.add)
            vec.tensor_tensor(out=tev, in0=uev[:, 1], in1=t_out, op=AL.mult)
            vec.tensor_tensor(out=tev, in0=tev, in1=uev[:, 0], op=AL.add)
            clamp(st1v[:, :, :, par], tev)

        # even: s = (c0 + c8*X) + Xr*(c2 + c10*X)   (ce = [c0,c2,c8,c10])
        l0_half(0, ce, bv(X), bv(Xr))
        vec.tensor_copy(out=Xrc[:], in_=pXrc[:])
        # odd:  s = (c0 + c4*Xc) + Xrc*(c1 + c5*Xc) (co = [c0,c1,c4,c5])
        l0_half(1, co, bv(Xc), bv(Xrc))

        # ---- generic layer evaluation (A-first), returns pre-clamp AP ----
        u = pwk.tile([P, 8 * B * Y], DT, tag="u")
        v_t = pwk.tile([P, 4 * B * Y], DT, tag="v")
        w2 = pwk.tile([P, 2 * B * Y], DT, tag="w2")
        tt = pwk.tile([P, B * Y], DT, tag="tt")

        def eval_layer(cv, tA, tB_, tC, tD, w):
            # cv: (p, two, i, w) coeff view; taps: (p, B, w) APs
            cHI, cLO = cv[:, 1], cv[:, 0]
            uv = u[:, : 8 * B * w].rearrange("p (i b y) -> p i b y", i=8, b=B)
            vec.tensor_tensor(out=uv, in0=bcast_c(cHI), in1=bcast_t(tA, 8), op=AL.mult)
            vec.tensor_tensor(out=uv, in0=uv, in1=bcast_c(cLO), op=AL.add)
            uc = u[:, : 8 * B * w].rearrange(
                "p (two j b y) -> p two j b y", two=2, j=4, b=B
            )
            vv = v_t[:, : 4 * B * w].rearrange("p (j b y) -> p j b y", j=4, b=B)
            vec.tensor_tensor(out=vv, in0=uc[:, 1], in1=bcast_t(tB_, 4), op=AL.mult)
            vec.tensor_tensor(out=vv, in0=vv, in1=uc[:, 0], op=AL.add)
            vc = v_t[:, : 4 * B * w].rearrange(
                "p (two j b y) -> p two j b y", two=2, j=2, b=B
            )
            wv = w2[:, : 2 * B * w].rearrange("p (j b y) -> p j b y", j=2, b=B)
            vec.tensor_tensor(out=wv, in0=vc[:, 1], in1=bcast_t(tC, 2), op=AL.mult)
            vec.tensor_tensor(out=wv, in0=wv, in1=vc[:, 0], op=AL.add)
            wc = w2[:, : 2 * B * w].rearrange("p (two b y) -> p two b y", two=2, b=B)
            tv = tt[:, : B * w].rearrange("p (b y) -> p b y", b=B)
            vec.tensor_tensor(out=tv, in0=wc[:, 1], in1=tD, op=AL.mult)
            vec.tensor_tensor(out=tv, in0=tv, in1=wc[:, 0], op=AL.add)
            return tv

        def rowshifted(src, n, tag):
            # PE permutation matmul + ScalarE copy-back; returns SBUF tile
            pt = pps.tile([P, n], F32, tag="p" + tag)
            ten.matmul(pt[:], psh[:], src[:], start=True, stop=True)
            out = pst.tile([P, n], DT, tag=tag)
            act.copy(out=out[:], in_=pt[:])
            return out

        # ---- layers 1, 2 ----
        st = st1
        bvy = lambda t: t[:].rearrange("p (b y) -> p b y", b=B)
        for l in (1, 2):
            if l == 1:
                # bits commute: do s=1,2,4 within each 8-map half (each only
                # needs its half's sigmoid), then s=8 across halves
                for h in (0, 1):
                    blk = twl(1)[:, h * 8 * Y : (h + 1) * 8 * Y]
                    for s in (1, 2, 4):
                        vh = blk.rearrange(
                            "p (hi two lo y) -> p hi two lo y", hi=4 // s, two=2, lo=s
                        )
                        vec.tensor_tensor(
                            out=vh[:, :, 1], in0=vh[:, :, 1], in1=vh[:, :, 0],
                            op=AL.subtract,
                        )
                v8 = twl(1).rearrange("p (two lo y) -> p two lo y", two=2, lo=8)
                vec.tensor_tensor(
                    out=v8[:, 1], in0=v8[:, 1], in1=v8[:, 0], op=AL.subtract
                )
            else:
                mobius_full(twl(l), Y)
            sB = pst.tile([P, B * Y], DT, tag="sB")
            colshift(sB, st, Y, [sync, nc.scalar])
            sC = rowshifted(st, B * Y, "sC")
            sD = rowshifted(sB, B * Y, "sD")
            cv = twl(l).rearrange("p (two i y) -> p two i y", two=2, i=8)
            tv = eval_layer(cv, bvy(st), bvy(sB), bvy(sC), bvy(sD), Y)
            if l == 1:
                stn = pst.tile([P, B * Y], DT, tag="state")
                clamp(bvy(stn), tv)
            else:
                # layer-3 state stored as parity planes: [even b*k | odd b*k]
                stn = pst.tile([P, B * Y], DT, tag="state")
                tvp = tv.rearrange("p b (k t) -> p b k t", t=2)
                clamp(bv(stn[:, 0 : B * K]), tvp[:, :, :, 0])
                clamp(bv(stn[:, B * K : 2 * B * K]), tvp[:, :, :, 1])
            st = stn

        # ---- layer 3 (even outputs only; compact coeffs, plane taps) ----
        mobius_full(twl(3)[:, 0 : M * K], K)
        sC = rowshifted(st, B * Y, "sC")
        out_t = pwk.tile([P, B * K], F32, tag="out")
        cv3 = twl(3)[:, 0 : M * K].rearrange("p (two i k) -> p two i k", two=2, i=8)
        tv = eval_layer(
            cv3,
            bv(st[:, 0 : B * K]),
            bv(st[:, B * K : 2 * B * K]),
            bv(sC[:, 0 : B * K]),
            bv(sC[:, B * K : 2 * B * K]),
            K,
        )
        # split the output clamp+store so the first DMA overlaps the rest
        ov = out_t[:].rearrange("p (b k) -> p b k", b=B)
        oda = out_ap_d.transpose([1, 0, 2])
        h = B // 2
        clamp(ov[:, 0:h], tv[:, 0:h])
        sync.dma_start(out=oda[:, 0:h], in_=ov[:, 0:h])
        clamp(ov[:, h:B], tv[:, h:B])
        nc.scalar.dma_start(out=oda[:, h:B], in_=ov[:, h:B])


_NC_CACHE = {}


def _np_dt():
    return {F32: np.float32, mybir.dt.float16: np.float16}[DT]


def build():
    if "nc" in _NC_CACHE:
        return _NC_CACHE["nc"]
    nc = bacc.Bacc(
        "TRN2",
        target_bir_lowering=False,
        debug=False,
        enable_asserts=False,
        num_devices=N_CORES,
    )
    x_d = nc.dram_tensor("x", (B, P, K), F32, kind="ExternalInput")
    tg_d = nc.dram_tensor("tg", (L, M, P, Y), F32, kind="ExternalInput")
    ps_d = nc.dram_tensor("pshift", (P, P), DT, kind="ExternalInput")
    out_d = nc.dram_tensor("out", (B, P, K), F32, kind="ExternalOutput")
    with TileContext(nc) as tc:
        _emit(tc, nc, x_d.ap(), tg_d.ap(), ps_d.ap(), out_d.ap())
    nc.compile()
    _NC_CACHE["nc"] = nc
    return nc


def _pshift():
    p = np.eye(P, k=-1, dtype=np.float64)
    p[0, P - 1] = 1.0
    return p.astype(_np_dt())


def make_in_maps(x, toggle_gates):
    x = np.ascontiguousarray(np.asarray(x, dtype=np.float32))
    tg = np.ascontiguousarray(np.asarray(toggle_gates, dtype=np.float32))
    psm = _pshift()
    return [
        {"x": x[c * B : (c + 1) * B], "tg": tg, "pshift": psm}
        for c in range(N_CORES)
    ]


def kernel(x, toggle_gates):
    nc = build()
    res = run_bass_kernel_spmd(
        nc, make_in_maps(x, toggle_gates), core_ids=list(range(N_CORES))
    )
    return np.concatenate([res.results[c]["out"] for c in range(N_CORES)], axis=0)


# revision 28
# speedup vs baseline: 1.1929x; 1.1929x over previous
"""Trainium2 Bass kernel for the soft-logic-gate CA problem.

Math (per sample, grid 128x128, 4 layers):
  state' = clip( sum_m sigmoid(tg[l,m]) * prod_j g(bit_j(m), tap_j), 0, 1 )
  taps: A=state[x,y], B=state[x,y+1], C=state[x+1,y], D=state[x+1,y+1] (periodic)
  g(0,t)=1-t, g(1,t)=t;  m = bA*8 + bB*4 + bC*2 + bD.

This is 4-D multilinear interpolation of the 16 gate maps at corner
(A,B,C,D).  Sigmoided gates are converted to multilinear-polynomial
coefficients with an in-place Moebius transform (c[m] -= c[m-bit]) and
each layer is evaluated with a Horner butterfly of fused tensor_tensor
ops, contracting A first (its tap needs no shift), then B, C, D:
  u_i = c[i] + c[8+i]*A ; v_j = u_j + u_{4+j}*B ; w_k = ... ; s = w0 + w1*D

Sharding: batch 32 -> 8 cores x 4 samples (gates replicated).
Layout per core: partition = grid row (128), free = (sample b:4, col y:128).
Coefficients stay compact (one copy); batch broadcast via stride-0 APs.
Row shifts (x+1) run on the idle TensorE as a permutation matmul into
PSUM, then ScalarE copies back to SBUF; col shifts (y+1) are cheap
same-partition DMAs.  GpSimd is left idle on purpose: its SBUF port is
shared with VectorE and concurrent use slows DVE ~4-6x (measured).

Compute dtype is fp16 (DVE 2x_1P mode on unit-stride ops; ~1.7e-3 rel
err vs fp32 reference, measured).  Layer 0 reads only 8 gate maps (the
initial state has x in even columns, 0 in odd), and layer 3 computes
only even output columns; both use de-interleaved compact coefficient
blocks so every hot op keeps unit stride.
"""

import numpy as np

import concourse.bacc as bacc
import concourse.mybir as mybir
from concourse.tile import TileContext
from concourse.bass_utils import run_bass_kernel_spmd

F32 = mybir.dt.float32
DT = mybir.dt.float16  # compute dtype (float32 also works)
AL = mybir.AluOpType
P = 128          # partitions = grid rows
B = 4            # samples per core
Y = 128          # grid cols
K = 64           # x cols (even grid cols)
L = 4            # layers
M = 16           # gate combos
N_CORES = 8


def _emit(tc, nc, x_ap, tg_ap, ps_ap, out_ap_d):
    sync, vec, act, ten = nc.sync, nc.vector, nc.scalar, nc.tensor
    SIG = mybir.ActivationFunctionType.Sigmoid

    def colshift(dst, src, w, engines):
        # per sample block of width w: dst[., b, y] = src[., b, (y+1) % w]
        d = dst[:].rearrange("p (b y) -> p b y", b=B)
        s = src[:].rearrange("p (b y) -> p b y", b=B)
        engines[0].dma_start(out=d[:, :, 0 : w - 1], in_=s[:, :, 1:w])
        engines[1].dma_start(out=d[:, :, w - 1 : w], in_=s[:, :, 0:1])

    def bcast_c(c):  # coeff (p, n, w) -> (p, n, B, w)
        n, w = c.shape[1], c.shape[2]
        return c.unsqueeze(2).broadcast_to((P, n, B, w))

    def bcast_t(t, n):  # tap (p, B, w) -> (p, n, B, w)
        w = t.shape[2]
        return t.unsqueeze(1).broadcast_to((P, n, B, w))

    def clamp(out_ap, in_ap):
        vec.tensor_scalar(
            out=out_ap, in0=in_ap, scalar1=0.0, scalar2=1.0, op0=AL.max, op1=AL.min
        )

    with (
        tc.tile_pool(name="coef", bufs=1) as pc,
        tc.tile_pool(name="io", bufs=2) as pio,
        tc.tile_pool(name="st", bufs=2) as pst,
        tc.tile_pool(name="wk", bufs=1) as pwk,
        tc.tile_pool(name="ps", bufs=2, space="PSUM") as pps,
    ):
        # preload the sigmoid ACT table while DMAs run
        scr = pwk.tile([P, 1], F32, tag="scr")
        vec.memset(scr[:], 0.0)
        act.activation(out=scr[:], in_=scr[:], func=SIG)

        # ---- loads (tg0 first: it gates the layer-0 coefficient chain) ----
        tw = pc.tile([P, L * M * Y], DT, tag="tw")  # coeffs, all layers
        tga = tg_ap

        def twl(l):
            return tw[:, l * M * Y : (l + 1) * M * Y]

        # layer 0 needs only 8 gate maps: {0,2,8,10} (even-y outputs) and
        # {0,1,4,5} (odd-y outputs); m = a*8 + bb*4 + c*2 + d.
        # tgraw0 = [4 even-set maps (a,c) | 4 odd-set maps (bb,d)] x y
        tgraw0 = pio.tile([P, 8 * Y], F32, tag="tgraw0")
        tg0t = tga[0].transpose([1, 0, 2])  # (P, M, Y)
        tg0m = tg0t.rearrange("p (a bb c d) y -> p a bb c d y", a=2, bb=2, c=2, d=2)
        g0e = tgraw0[:, 0 : 4 * Y].rearrange("p (a c y) -> p a c y", a=2, c=2)
        g0o = tgraw0[:, 4 * Y : 8 * Y].rearrange("p (bb d y) -> p bb d y", bb=2, d=2)
        for i in (0, 1):  # DMA APs allow at most partition + 3 dims
            sync.dma_start(out=g0e[:, i], in_=tg0m[:, i, 0, :, 0, :])
        for i in (0, 1):
            sync.dma_start(out=g0o[:, i], in_=tg0m[:, 0, i, 0, :, :])

        # layer-0 taps, loaded straight from DRAM with casting gpsimd DMAs:
        # X, Xc = colshift(X) (as rotated DRAM slices); Xr/Xrc via PE rowshift
        xt = x_ap.transpose([1, 0, 2])  # (P, B, K)
        X = pwk.tile([P, B * K], DT, tag="X")
        Xc = pwk.tile([P, B * K], DT, tag="Xc")
        Xv = X[:].rearrange("p (b k) -> p b k", b=B)
        Xcv = Xc[:].rearrange("p (b k) -> p b k", b=B)
        nc.gpsimd.dma_start(out=Xv, in_=xt)
        nc.gpsimd.dma_start(out=Xcv[:, :, 0 : K - 1], in_=xt[:, :, 1:K])
        nc.gpsimd.dma_start(out=Xcv[:, :, K - 1 : K], in_=xt[:, :, 0:1])
        psh = pwk.tile([P, P], DT, tag="psh")
        nc.scalar.dma_start(out=psh[:], in_=ps_ap)
        pXr = pps.tile([P, B * K], F32, tag="pXr")
        pXrc = pps.tile([P, B * K], F32, tag="pXrc")
        ten.matmul(pXr[:], psh[:], X[:], start=True, stop=True)
        ten.matmul(pXrc[:], psh[:], Xc[:], start=True, stop=True)
        Xr = pwk.tile([P, B * K], DT, tag="Xr")
        Xrc = pwk.tile([P, B * K], DT, tag="Xrc")
        vec.tensor_copy(out=Xr[:], in_=pXr[:])
        vec.tensor_copy(out=Xrc[:], in_=pXrc[:])

        # ---- layer-0 coefficients: two compact 4-map blocks ----
        #   ce = twl0[0:256]   = [c0,c2,c8,c10] x k    (order (a,c))
        #   co = twl0[256:512] = [c0,c1,c4,c5]  x k    (order (bb,d))
        t0e = g0e.rearrange("p a c (k t) -> p a c k t", t=2)
        t0o = g0o.rearrange("p bb d (k t) -> p bb d k t", t=2)
        ce = twl(0)[:, 0:256]
        co = twl(0)[:, 256:512]
        ce4 = ce.rearrange("p (a c k) -> p a c k", a=2, c=2)
        co4 = co.rearrange("p (bb d k) -> p bb d k", bb=2, d=2)
        act.activation(out=ce4, in_=t0e[:, :, :, :, 0], func=SIG)
        act.activation(out=co4, in_=t0o[:, :, :, :, 1], func=SIG)
        # 2-D Moebius on each block (2 fused subtract passes each)
        for blk, n4 in ((ce, ce4), (co, co4)):
            d_ = n4[:, :, 1]
            s_ = n4[:, :, 0]
            vec.tensor_tensor(out=d_, in0=d_, in1=s_, op=AL.subtract)
            hi = blk.rearrange("p (h q) -> p h q", h=2)
            vec.tensor_tensor(
                out=hi[:, 1], in0=hi[:, 1], in1=hi[:, 0], op=AL.subtract
            )

        # remaining layers: full sigmoid (layer 3 de-interleaved to even-y)
        for l in (1, 2, 3):
            tgraw = pio.tile([P, M * Y], F32, tag="tgraw")
            sync.dma_start(
                out=tgraw[:].rearrange("p (m y) -> p m y", m=M),
                in_=tga[l].transpose([1, 0, 2]),
            )
            if l < 3:
                act.activation(out=twl(l), in_=tgraw[:], func=SIG)
            else:
                tr = tgraw[:].rearrange("p (m k t) -> p m k t", m=M, t=2)
                c3 = twl(3)[:, 0 : M * K].rearrange("p (m k) -> p m k", m=M)
                act.activation(out=c3, in_=tr[:, :, :, 0], func=SIG)

        def mobius_full(block, w):
            # block: (p, 16*w) coeff AP; in-place c[m] -= c[m-bit] per bit
            for s in (1, 2, 4, 8):
                hi = 8 // s
                v = block.rearrange(
                    "p (hi two lo y) -> p hi two lo y", hi=hi, two=2, lo=s, y=w
                )
                vec.tensor_tensor(
                    out=v[:, :, 1], in0=v[:, :, 1], in1=v[:, :, 0], op=AL.subtract
                )

        # ---- layer 0 eval (A-first 2-D interp) ----
        st1 = pst.tile([P, B * Y], DT, tag="state")
        st1v = st1[:].rearrange("p (b k t) -> p b k t", b=B, t=2)
        ue = pwk.tile([P, 2 * B * K], DT, tag="ue")
        te = pwk.tile([P, B * K], DT, tag="te")
        uev = ue[:].rearrange("p (s b k) -> p s b k", s=2, b=B)
        tev = te[:].rearrange("p (b k) -> p b k", b=B)
        bv = lambda t: t[:].rearrange("p (b k) -> p b k", b=B)

        def l0_half(par, cpair, t_in, t_out):
            # s = (c_lo0 + c_hi0 * t_in) + t_out * (c_lo1 + c_hi1 * t_in)
            cp = cpair.rearrange("p (h s k) -> p h s k", h=2, s=2)
            vec.tensor_tensor(out=uev, in0=bcast_c(cp[:, 1]), in1=bcast_t(t_in, 2), op=AL.mult)
            vec.tensor_tensor(out=uev, in0=uev, in1=bcast_c(cp[:, 0]), op=AL.add)
            vec.tensor_tensor(out=tev, in0=uev[:, 1], in1=t_out, op=AL.mult)
            vec.tensor_tensor(out=tev, in0=tev, in1=uev[:, 0], op=AL.add)
            clamp(st1v[:, :, :, par], tev)

        # even: s = (c0 + c8*X) + Xr*(c2 + c10*X)   (ce = [c0,c2,c8,c10])
        l0_half(0, ce, bv(X), bv(Xr))
        # odd:  s = (c0 + c4*Xc) + Xrc*(c1 + c5*Xc) (co = [c0,c1,c4,c5])
        l0_half(1, co, bv(Xc), bv(Xrc))

        # ---- generic layer evaluation (A-first), returns pre-clamp AP ----
        u = pwk.tile([P, 8 * B * Y], DT, tag="u")
        v_t = pwk.tile([P, 4 * B * Y], DT, tag="v")
        w2 = pwk.tile([P, 2 * B * Y], DT, tag="w2")
        tt = pwk.tile([P, B * Y], DT, tag="tt")

        def eval_layer(cv, tA, tB_, tC, tD, w):
            # cv: (p, two, i, w) coeff view; taps: (p, B, w) APs
            cHI, cLO = cv[:, 1], cv[:, 0]
            uv = u[:, : 8 * B * w].rearrange("p (i b y) -> p i b y", i=8, b=B)
            vec.tensor_tensor(out=uv, in0=bcast_c(cHI), in1=bcast_t(tA, 8), op=AL.mult)
            vec.tensor_tensor(out=uv, in0=uv, in1=bcast_c(cLO), op=AL.add)
            uc = u[:, : 8 * B * w].rearrange(
                "p (two j b y) -> p two j b y", two=2, j=4, b=B
            )
            vv = v_t[:, : 4 * B * w].rearrange("p (j b y) -> p j b y", j=4, b=B)
            vec.tensor_tensor(out=vv, in0=uc[:, 1], in1=bcast_t(tB_, 4), op=AL.mult)
            vec.tensor_tensor(out=vv, in0=vv, in1=uc[:, 0], op=AL.add)
            vc = v_t[:, : 4 * B * w].rearrange(
                "p (two j b y) -> p two j b y", two=2, j=2, b=B
            )
            wv = w2[:, : 2 * B * w].rearrange("p (j b y) -> p j b y", j=2, b=B)
            vec.tensor_tensor(out=wv, in0=vc[:, 1], in1=bcast_t(tC, 2), op=AL.mult)
            vec.tensor_tensor(out=wv, in0=wv, in1=vc[:, 0], op=AL.add)
            wc = w2[:, : 2 * B * w].rearrange("p (two b y) -> p two b y", two=2, b=B)
            tv = tt[:, : B * w].rearrange("p (b y) -> p b y", b=B)
            vec.tensor_tensor(out=tv, in0=wc[:, 1], in1=tD, op=AL.mult)
            vec.tensor_tensor(out=tv, in0=tv, in1=wc[:, 0], op=AL.add)
            return tv

        def rowshifted(src, n, tag):
            # PE permutation matmul + ScalarE copy-back; returns SBUF tile
            pt = pps.tile([P, n], F32, tag="p" + tag)
            ten.matmul(pt[:], psh[:], src[:], start=True, stop=True)
            out = pst.tile([P, n], DT, tag=tag)
            act.copy(out=out[:], in_=pt[:])
            return out

        # ---- layers 1, 2 ----
        st = st1
        bvy = lambda t: t[:].rearrange("p (b y) -> p b y", b=B)
        for l in (1, 2):
            mobius_full(twl(l), Y)
            sB = pst.tile([P, B * Y], DT, tag="sB")
            colshift(sB, st, Y, [sync, nc.scalar])
            sC = rowshifted(st, B * Y, "sC")
            sD = rowshifted(sB, B * Y, "sD")
            cv = twl(l).rearrange("p (two i y) -> p two i y", two=2, i=8)
            tv = eval_layer(cv, bvy(st), bvy(sB), bvy(sC), bvy(sD), Y)
            if l == 1:
                stn = pst.tile([P, B * Y], DT, tag="state")
                clamp(bvy(stn), tv)
            else:
                # layer-3 state stored as parity planes: [even b*k | odd b*k]
                stn = pst.tile([P, B * Y], DT, tag="state")
                tvp = tv.rearrange("p b (k t) -> p b k t", t=2)
                clamp(bv(stn[:, 0 : B * K]), tvp[:, :, :, 0])
                clamp(bv(stn[:, B * K : 2 * B * K]), tvp[:, :, :, 1])
            st = stn

        # ---- layer 3 (even outputs only; compact coeffs, plane taps) ----
        mobius_full(twl(3)[:, 0 : M * K], K)
        sC = rowshifted(st, B * Y, "sC")
        out_t = pwk.tile([P, B * K], F32, tag="out")
        cv3 = twl(3)[:, 0 : M * K].rearrange("p (two i k) -> p two i k", two=2, i=8)
        tv = eval_layer(
            cv3,
            bv(st[:, 0 : B * K]),
            bv(st[:, B * K : 2 * B * K]),
            bv(sC[:, 0 : B * K]),
            bv(sC[:, B * K : 2 * B * K]),
            K,
        )
        # split the output clamp+store so the first DMA overlaps the rest
        ov = out_t[:].rearrange("p (b k) -> p b k", b=B)
        oda = out_ap_d.transpose([1, 0, 2])
        h = B // 2
        clamp(ov[:, 0:h], tv[:, 0:h])
        sync.dma_start(out=oda[:, 0:h], in_=ov[:, 0:h])
        clamp(ov[:, h:B], tv[:, h:B])
        nc.scalar.dma_start(out=oda[:, h:B], in_=ov[:, h:B])


_NC_CACHE = {}


def _np_dt():
    return {F32: np.float32, mybir.dt.float16: np.float16}[DT]


def build():
    if "nc" in _NC_CACHE:
        return _NC_CACHE["nc"]
    nc = bacc.Bacc(
        "TRN2",
        target_bir_lowering=False,
        debug=False,
        enable_asserts=False,
        num_devices=N_CORES,
    )
    x_d = nc.dram_tensor("x", (B, P, K), F32, kind="ExternalInput")
    tg_d = nc.dram_tensor("tg", (L, M, P, Y), F32, kind="ExternalInput")
    ps_d = nc.dram_tensor("pshift", (P, P), DT, kind="ExternalInput")
    out_d = nc.dram_tensor("out", (B, P, K), F32, kind="ExternalOutput")
    with TileContext(nc) as tc:
        _emit(tc, nc, x_d.ap(), tg_d.ap(), ps_d.ap(), out_d.ap())
    nc.compile()
    _NC_CACHE["nc"] = nc
    return nc


def _pshift():
    p = np.eye(P, k=-1, dtype=np.float64)
    p[0, P - 1] = 1.0
    return p.astype(_np_dt())


def make_in_maps(x, toggle_gates):
    x = np.ascontiguousarray(np.asarray(x, dtype=np.float32))
    tg = np.ascontiguousarray(np.asarray(toggle_gates, dtype=np.float32))
    psm = _pshift()
    return [
        {"x": x[c * B : (c + 1) * B], "tg": tg, "pshift": psm}
        for c in range(N_CORES)
    ]


def kernel(x, toggle_gates):
    nc = build()
    res = run_bass_kernel_spmd(
        nc, make_in_maps(x, toggle_gates), core_ids=list(range(N_CORES))
    )
    return np.concatenate([res.results[c]["out"] for c in range(N_CORES)], axis=0)
